# revision 42
# baseline (speedup 1.0000x reference)
"""Local causal (sliding-window) attention block on 8 TRN2 NeuronCores.

Reference computation (per batch b):
    h = LayerNorm(x) * gamma + beta
    Q = h@Wq, K = h@Wk, V = h@Wv          (heads: 16 x 64)
    S = QK^T/sqrt(dk) masked to causal band of width 256
    out = x + softmax(S)@V @ Wo + bo

Sharding: 8 cores = 2 batches x 4 head-groups (4 heads each).
Each core computes LN(x_b), its head-group's Q/K/V, banded attention,
and a partial out-projection  attn_g @ Wo[g]  (token-major, [T, D]).
Host reduces: out[b] = x[b] + sum_g partial[b,g] + bo.

Implementation notes (v2, overhead-optimized):
- All matmuls run in bf16 (fp32 PSUM accumulation).
- h^T is produced by the DMA XBAR transpose (dma_start(transpose=True)),
  eliminating all PE transposes.
- Attention computes S^T[k, q] tiles directly (k on partitions), so the
  probability tiles feed P@V without any transpose; the softmax
  denominator comes from a ones-column appended to V, and the final
  1/den scaling uses a stride-0 DMA broadcast + one Pool multiply.
- Elementwise work is spread across DVE / Act / Pool to keep the PE the
  only near-saturated engine.
"""

import os

import numpy as np

import concourse.bass as bass
import concourse.tile as tile
from concourse import bacc, mybir
from concourse.bass_utils import run_bass_kernel_spmd

F32 = mybir.dt.float32
F32R = mybir.dt.float32r
BF16 = mybir.dt.bfloat16

T = 2048          # tokens per batch
D = 1024          # model dim
HG = 4            # heads per core
DK = 64           # head dim
DG = HG * DK      # head-group feature width (256)
WIN = 256         # attention window
P = 128           # partitions
NT = T // P       # 16 token tiles
KC = D // P       # 8 feature chunks
NG = NT // 4      # 4 query groups of 512 tokens
LN_EPS = 1e-5
MASKVAL = -1e9

# filled by test.py via run(trace=True)
LAST_PROFILE = {}


def _nq(kb):
    return min(3 * P, (NT - kb) * P)


def _body(tc):
    nc = tc.nc

    x = nc.dram_tensor("x", [T, D], BF16, kind="ExternalInput").ap()
    wq = nc.dram_tensor("wq", [P, KC, DG], BF16, kind="ExternalInput").ap()
    wk = nc.dram_tensor("wk", [P, KC, DG], BF16, kind="ExternalInput").ap()
    wv = nc.dram_tensor("wv", [P, KC, DG], BF16, kind="ExternalInput").ap()
    wo = nc.dram_tensor("wo", [P, DG // P, D], BF16, kind="ExternalInput").ap()
    bq = nc.dram_tensor("bq", [P, DG // P], F32, kind="ExternalInput").ap()
    bk = nc.dram_tensor("bk", [P, DG // P], F32, kind="ExternalInput").ap()
    mc = nc.dram_tensor("mc", [P, HG, 2 * P], BF16, kind="ExternalInput").ap()
    zz = nc.dram_tensor("zz", [1, DK + 1 + 512], BF16, kind="ExternalInput").ap()
    cr = nc.dram_tensor("cr", [1, P + DG], BF16, kind="ExternalInput").ap()
    vones = nc.dram_tensor("vones", [P, NT, HG], BF16, kind="ExternalInput").ap()
    partial = nc.dram_tensor("partial", [T, D], BF16, kind="ExternalOutput").ap()
    dbg = os.environ.get("KDEBUG", "") == "1"
    dscr = nc.dram_tensor("dscr", [NG * 2, 1024], F32,
                          kind="ExternalOutput" if dbg else "Internal").ap()
    dscr2 = nc.dram_tensor("dscr2", [NG * 2, 1024], F32, kind="Internal").ap()
    if dbg:
        d_ht = nc.dram_tensor("d_ht", [P, KC, T], BF16, kind="ExternalOutput").ap()
        d_qt = nc.dram_tensor("d_qt", [P, 2, T], BF16, kind="ExternalOutput").ap()
        d_kt = nc.dram_tensor("d_kt", [P, 2, T], BF16, kind="ExternalOutput").ap()
        d_v = nc.dram_tensor("d_v", [P, NT, HG * (DK + 1)], BF16, kind="ExternalOutput").ap()
        d_ot = nc.dram_tensor("d_ot", [P, 2, T], BF16, kind="ExternalOutput").ap()

    with (
        tc.tile_pool(name="consts", bufs=1) as consts,
        tc.tile_pool(name="big", bufs=1) as big,
        tc.tile_pool(name="sp", bufs=2, space="PSUM") as sp,
        tc.tile_pool(name="ep", bufs=10) as ep,
    ):
        # ---- resident SBUF tensors ----
        wq_sb = consts.tile([P, KC, DG], BF16, tag="wq")
        wk_sb = consts.tile([P, KC, DG], BF16, tag="wk")
        wv_sb = consts.tile([P, KC, DG], BF16, tag="wv")
        wo_sb = consts.tile([P, DG // P, D], BF16, tag="wo")
        bq_sb = consts.tile([P, DG // P], F32, tag="bq")
        bk_sb = consts.tile([P, DG // P], F32, tag="bk")
        mc_sb = consts.tile([P, HG, 2 * P], BF16, tag="mc")
        zz_sb = consts.tile([1, DK + 1 + 512], BF16, tag="zz")
        cr_sb = consts.tile([1, P + DG], BF16, tag="cr")
        eps_sb = consts.tile([P, 1], F32, tag="eps")

        nc.vector.memset(eps_sb, LN_EPS)

        # h^T (feature-major), Q^T/K^T (feature-major), V (token-major,
        # with a ones column per head for the softmax denominator),
        # O^T (attention output, feature-major)
        ht_sb = big.tile([P, KC, T], BF16, tag="ht")
        qt_sb = big.tile([P, DG // P, T], BF16, tag="qt")
        kt_sb = big.tile([P, DG // P, T], BF16, tag="kt")
        v_sb = big.tile([P, NT, HG * (DK + 1)], BF16, tag="v")
        ot_sb = big.tile([P, DG // P, T], BF16, tag="ot")

        # ones columns of V (denominator trick)
        nc.sync.dma_start(out=v_sb[:, :, DK::DK + 1], in_=vones)

        # scores PSUM + exp-tile pools live across front and attention so
        # the first score quads can fill front-phase bubbles
        et_ref = {}

        def st_quad(kb):
            """S^T + exp + band mask for all 4 heads of key block kb."""
            nq = _nq(kb)
            ks = slice(kb * P, (kb + 1) * P)
            et = ep.tile([P, HG, 3 * P], BF16, tag="et")
            for j in range(2):
                s2 = sp.tile([P, 2, 512], F32, tag="s2")
                for hh in range(2):
                    h = 2 * j + hh
                    p0 = (h % 2) * DK
                    nc.tensor.matmul(
                        s2[:, hh, 0:nq],
                        kt_sb[p0:p0 + DK, h // 2, ks],
                        qt_sb[p0:p0 + DK, h // 2, kb * P:kb * P + nq],
                        start=True, stop=True,
                    )
                nc.scalar.activation(
                    out=et[:, 2 * j:2 * j + 2, 0:nq], in_=s2[:, :, 0:nq],
                    func=mybir.ActivationFunctionType.Exp,
                )
            # band mask: diag ([0:128)) and far ([256:384)) 0/1 windows
            nc.vector.tensor_mul(
                et[:, :, 0:P], et[:, :, 0:P], mc_sb[:, :, 0:P])
            if nq == 3 * P:
                nc.vector.tensor_mul(
                    et[:, :, 2 * P:3 * P], et[:, :, 2 * P:3 * P],
                    mc_sb[:, :, P:2 * P])
            et_ref[kb] = et

        # ============ Front: LayerNorm + h^T + Q/K/V projections ============
        x_sb = big.tile([P, NT, D], BF16, tag="x")
        xr = x.rearrange("(n p) d -> p n d", p=P)
        nc.sync.dma_start(out=x_sb[:, 0:2, :], in_=xr[:, 0:2, :])
        nc.sync.dma_start(out=wv_sb, in_=wv)
        nc.sync.dma_start(out=cr_sb, in_=cr)
        nc.sync.dma_start(out=x_sb[:, 2:4, :], in_=xr[:, 2:4, :])
        nc.sync.dma_start(out=wq_sb, in_=wq)
        nc.sync.dma_start(out=wk_sb, in_=wk)
        nc.sync.dma_start(out=bq_sb, in_=bq)
        nc.sync.dma_start(out=bk_sb, in_=bk)
        nc.sync.dma_start(out=mc_sb, in_=mc)
        nc.sync.dma_start(out=zz_sb, in_=zz)
        for q in range(2, 8):
            nc.sync.dma_start(
                out=x_sb[:, 2 * q:2 * (q + 1), :], in_=xr[:, 2 * q:2 * (q + 1), :])
        nc.sync.dma_start(out=wo_sb, in_=wo)
        with (
            tc.tile_pool(name="hp", bufs=3) as hp,
            tc.tile_pool(name="lnst", bufs=4) as lnst,
            tc.tile_pool(name="qkp", bufs=2, space="PSUM") as qkp,
            tc.tile_pool(name="vp", bufs=2, space="PSUM") as vp,
        ):
            front_state = {}
            for tb in range(NT):
                xt = x_sb[:, tb, :]

                j4 = tb % 2
                if j4 == 0:
                    mv4 = lnst.tile([P, 2, 2], F32, tag="mv4")
                    rstd4 = lnst.tile([P, 2], F32, tag="rstd4")
                    front_state.update(mv4=mv4, rstd4=rstd4)
                mv4 = front_state["mv4"]
                rstd4 = front_state["rstd4"]
                stats = lnst.tile([P, 2, 6], F32, tag="stats")
                xg = xt.rearrange("p (g d) -> p g d", g=2)
                nc.vector.bn_stats(out=stats[:, 0, :], in_=xg[:, 0, :])
                nc.vector.bn_stats(out=stats[:, 1, :], in_=xg[:, 1, :])
                nc.vector.bn_aggr(out=mv4[:, j4, :], in_=stats)
                if j4 == 1:
                    # rstd = exp(-0.5*ln(var+eps)), batched over 4 tiles
                    nc.scalar.activation(
                        out=rstd4, in_=mv4[:, :, 1],
                        func=mybir.ActivationFunctionType.Ln,
                        bias=eps_sb, scale=1.0,
                    )
                    nc.scalar.activation(
                        out=rstd4, in_=rstd4,
                        func=mybir.ActivationFunctionType.Exp,
                        scale=-0.5,
                    )
                    for t2 in range(tb - 1, tb + 1):
                        j2 = t2 % 2
                        rs1 = lnst.tile([P, 1], F32, tag="rs1")
                        nc.vector.tensor_copy(rs1, rstd4[:, j2:j2 + 1])
                        nm1 = lnst.tile([P, 1], F32, tag="nm1")
                        nc.vector.tensor_scalar(
                            out=nm1, in0=mv4[:, j2, 0:1], scalar1=rs1,
                            scalar2=-1.0, op0=mybir.AluOpType.mult,
                            op1=mybir.AluOpType.mult,
                        )
                        hn = hp.tile([P, D], BF16, tag="hn")
                        nc.gpsimd.tensor_scalar(
                            out=hn, in0=x_sb[:, t2, :], scalar1=rs1,
                            scalar2=nm1, op0=mybir.AluOpType.mult,
                            op1=mybir.AluOpType.add,
                        )
                        nc.sync.dma_start(
                            out=ht_sb[:, :, t2 * P:(t2 + 1) * P], in_=hn,
                            transpose=True)

                if tb % 4 != 3:
                    continue

                # V projection for the 4 finished tiles (token-major);
                # bv enters as a rank-1 (ones x bv) accumulation
                for t2 in range(tb - 3, tb + 1):
                    t2s = slice(t2 * P, (t2 + 1) * P)
                    ps = vp.tile([P, DG], F32, tag="psv")
                    for kc in range(KC):
                        nc.tensor.matmul(
                            ps, ht_sb[:, kc, t2s], wv_sb[:, kc, :],
                            start=(kc == 0), stop=False,
                        )
                    nc.tensor.matmul(
                        ps, cr_sb[0:1, 0:P], cr_sb[0:1, P:],
                        start=False, stop=True,
                    )
                    nc.scalar.activation(
                        out=v_sb[:, t2, :].rearrange(
                            "p (h d) -> p h d", d=DK + 1)[:, :, 0:DK],
                        in_=ps.rearrange("p (h d) -> p h d", d=DK),
                        func=mybir.ActivationFunctionType.Identity, scale=1.0)

                # Q^T / K^T per completed 512-token slice
                if True:
                    sl = tb // 4
                    ss = slice(sl * 512, (sl + 1) * 512)
                    for w_sb, b_sb, dst in ((wq_sb, bq_sb, qt_sb),
                                            (wk_sb, bk_sb, kt_sb)):
                        for oc in range(DG // P):
                            pq = qkp.tile([P, 512], F32, tag="psqk")
                            for kc in range(KC):
                                nc.tensor.matmul(
                                    pq,
                                    w_sb[:, kc, oc * P:(oc + 1) * P],
                                    ht_sb[:, kc, ss],
                                    start=(kc == 0), stop=(kc == KC - 1),
                                )
                            nc.vector.tensor_scalar_add(
                                dst[:, oc, ss], pq, b_sb[:, oc:oc + 1])

        # ============ Attention (S^T formulation) + out-projection ============
        with (
            tc.tile_pool(name="pvop", bufs=4, space="PSUM") as pvop,
            tc.tile_pool(name="rp", bufs=4) as rp,
            tc.tile_pool(name="bp", bufs=3) as bp,
            tc.tile_pool(name="op", bufs=3) as op,
        ):
            def pv_head(g, h, den2, j):
                """P@V for one head; den row lands in half j of den2."""
                q0 = g * 512
                oc, hh = h // 2, h % 2
                av = pvop.tile([P, 512], F32, tag="po")
                nc.tensor.matmul(
                    av[0:DK + 1, :], zz_sb[0:1, 0:DK + 1], zz_sb[0:1, DK + 1:],
                    start=True, stop=False, skip_group_check=True,
                )
                segs = []
                for kb in range(max(0, 4 * g - 2), 4 * g + 4):
                    a = max(kb * P, q0)
                    b2 = min(kb * P + _nq(kb), q0 + 512)
                    segs.append((kb, a - q0, b2 - q0))
                for i, (kb, a, b2) in enumerate(segs):
                    nc.tensor.matmul(
                        av[0:DK + 1, a:b2],
                        v_sb[:, kb, h * (DK + 1):(h + 1) * (DK + 1)],
                        et_ref[kb][:, h, q0 + a - kb * P:q0 + b2 - kb * P],
                        start=False, stop=(i == len(segs) - 1),
                        skip_group_check=True,
                    )
                nc.scalar.activation(
                    out=den2[0:1, j * 512:(j + 1) * 512], in_=av[DK:DK + 1, :],
                    func=mybir.ActivationFunctionType.Identity, scale=1.0)
                return av

            def recip_pair(g, pair):
                """1/den for two heads: round-trip via DRAM so the
                reciprocal runs partition-parallel ([1,1024]->[128,8])."""
                i = g * 2 + pair
                den2 = recip_pair.den2
                w1 = nc.sync.dma_start(out=dscr[i:i + 1, :], in_=den2)
                tc.chain_iter_dep(f"dw{i}", w1.ins)
                dr = rp.tile([P, 8], F32, tag="dr")
                r1 = nc.sync.dma_start(
                    out=dr, in_=dscr[i:i + 1, :].rearrange(
                        "o (p j) -> (o p) j", p=P))
                tc.chain_iter_dep(f"dw{i}", r1.ins)
                nc.vector.reciprocal(out=dr, in_=dr)
                w2 = nc.sync.dma_start(
                    out=dscr2[i:i + 1, :].rearrange("o (p j) -> (o p) j", p=P),
                    in_=dr)
                tc.chain_iter_dep(f"db{i}", w2.ins)
                bc = bp.tile([DK, 2, 512], F32, tag="bc")
                r2 = nc.sync.dma_start(
                    out=bc, in_=dscr2[i:i + 1, :].rearrange(
                        "o (j q) -> o j q", j=2).to_broadcast([DK, 2, 512]))
                tc.chain_iter_dep(f"db{i}", r2.ins)
                return bc

            def norm_head(g, h, av, bc, j):
                q0 = g * 512
                oc, hh = h // 2, h % 2
                nc.vector.tensor_mul(
                    ot_sb[hh * DK:(hh + 1) * DK, oc, q0:q0 + 512],
                    av[0:DK, :], bc[:, j, :])

            def outproj(tb):
                ts = slice(tb * P, (tb + 1) * P)
                ob = op.tile([P, D], BF16, tag="ob")
                for on in range(2):
                    po = pvop.tile([P, 512], F32, tag="po")
                    for kd in range(DG // P):
                        nc.tensor.matmul(
                            po,
                            ot_sb[:, kd, ts],
                            wo_sb[:, kd, on * 512:(on + 1) * 512],
                            start=(kd == 0), stop=(kd == DG // P - 1),
                        )
                    if (tb + on) % 2 == 0:
                        nc.scalar.activation(
                            out=ob[:, on * 512:(on + 1) * 512], in_=po,
                            func=mybir.ActivationFunctionType.Identity,
                            scale=1.0)
                    else:
                        nc.vector.tensor_copy(
                            ob[:, on * 512:(on + 1) * 512], po)
                nc.sync.dma_start(out=partial[ts, :], in_=ob)

            for kb in range(4):
                st_quad(kb)
            for g in range(NG):
                nxt = [4 * (g + 1) + j for j in range(4)] if g + 1 < NG else []
                prv = list(range(4 * (g - 1), 4 * g)) if g > 0 else []
                den2a = rp.tile([1, 1024], F32, tag="den2")
                recip_pair.den2 = den2a
                av0 = pv_head(g, 0, den2a, 0)
                av1 = pv_head(g, 1, den2a, 1)
                bca = recip_pair(g, 0)
                if nxt:
                    st_quad(nxt[0])
                    st_quad(nxt[1])
                if prv:
                    outproj(prv[0])
                    outproj(prv[1])
                den2b = rp.tile([1, 1024], F32, tag="den2")
                recip_pair.den2 = den2b
                av2 = pv_head(g, 2, den2b, 0)
                av3 = pv_head(g, 3, den2b, 1)
                bcb = recip_pair(g, 1)
                if nxt:
                    st_quad(nxt[2])
                    st_quad(nxt[3])
                norm_head(g, 0, av0, bca, 0)
                norm_head(g, 1, av1, bca, 1)
                if prv:
                    outproj(prv[2])
                    outproj(prv[3])
                norm_head(g, 2, av2, bcb, 0)
                norm_head(g, 3, av3, bcb, 1)
            for tb in range(4 * (NG - 1), NT):
                outproj(tb)

            if dbg:
                nc.sync.dma_start(out=d_ht, in_=ht_sb)
                nc.sync.dma_start(out=d_qt, in_=qt_sb)
                nc.sync.dma_start(out=d_kt, in_=kt_sb)
                nc.sync.dma_start(out=d_v, in_=v_sb)
                nc.sync.dma_start(out=d_ot, in_=ot_sb)


def build_nc():
    nc = bacc.Bacc("TRN2", target_bir_lowering=False, debug=False,
                   num_devices=8)
    with tile.TileContext(nc) as tc:
        _body(tc)
    nc.compile()
    return nc


def _prep_core_inputs(x, Wq, Wk, Wv, Wo, gamma, beta):
    """Host-side prep: per-(batch, head-group) input dicts."""
    import ml_dtypes
    BF = ml_dtypes.bfloat16
    B = x.shape[0]
    kk = np.arange(P)[:, None]
    qq = np.arange(P)[None, :]
    md = (kk <= qq).astype(BF)
    mf = (kk > qq).astype(BF)
    m1 = np.concatenate([md, mf], axis=1)
    mcomb = np.ascontiguousarray(np.stack([m1] * 4, axis=1))

    def fold(w):
        # [D, DG] -> [128, KC, DG] with d = c*128 + p
        return np.ascontiguousarray(
            w.reshape(KC, P, DG).transpose(1, 0, 2)).astype(BF)

    in_maps = []
    for b in range(B):
        for g in range(4):
            sl = slice(g * DG, (g + 1) * DG)
            sq = np.float32(1.0 / np.sqrt(DK))
            wq_g = fold(gamma[:, None] * Wq[:, sl] * sq)
            wk_g = fold(gamma[:, None] * Wk[:, sl])
            wv_g = fold(gamma[:, None] * Wv[:, sl])
            wo_g = np.ascontiguousarray(
                Wo[sl, :].reshape(DG // P, P, D).transpose(1, 0, 2)).astype(BF)
            bq_g = ((beta @ Wq[:, sl]) * sq).astype(np.float32)
            bk_g = (beta @ Wk[:, sl]).astype(np.float32)
            bv_g = (beta @ Wv[:, sl]).astype(np.float32)
            crow = np.concatenate(
                [np.ones(P, dtype=np.float32), bv_g]).astype(BF)
            in_maps.append({
                "x": np.ascontiguousarray(x[b]).astype(BF),
                "wq": wq_g, "wk": wk_g, "wv": wv_g, "wo": wo_g,
                "bq": np.ascontiguousarray(bq_g.reshape(DG // P, P).T),
                "bk": np.ascontiguousarray(bk_g.reshape(DG // P, P).T),
                "cr": crow[None, :],
                "mc": mcomb,
                "zz": np.zeros((1, DK + 1 + 512), dtype=BF),
                "vones": np.ones((P, NT, HG), dtype=BF),
            })
    return in_maps


def _ntff_hook(so_path="/opt/axon/libaxon_pjrt.so"):
    import contextlib
    import ctypes

    lib = ctypes.CDLL(so_path)
    lib.axon_start_nrt_profile.argtypes = [
        ctypes.POINTER(ctypes.c_int64), ctypes.c_size_t]
    lib.axon_start_nrt_profile.restype = ctypes.c_int64
    lib.axon_stop_nrt_profile.argtypes = [ctypes.c_char_p]
    lib.axon_stop_nrt_profile.restype = ctypes.c_int64

    @contextlib.contextmanager
    def _hook(output_dir, device_ids):
        import jax
        jax.devices()
        if device_ids:
            ids = (ctypes.c_int64 * len(device_ids))(*device_ids)
            rc = lib.axon_start_nrt_profile(ids, len(device_ids))
        else:
            rc = lib.axon_start_nrt_profile(None, 0)
        if rc != 0:
            raise RuntimeError(f"axon_start_nrt_profile rc={rc}")
        try:
            yield
        finally:
            n = lib.axon_stop_nrt_profile(str(output_dir).encode())
            print(f"profile: {n} file(s) written to {output_dir}")

    return _hook


def _run_traced(nc, in_maps, trace_dir=None):
    """Execute via PJRT with NTFF capture; return BassKernelResults with
    exec_time_ns and a perfetto trace."""
    import glob
    import tempfile

    import gauge.profiler
    from concourse import bass2jax, bass_utils
    from concourse._compat import FishPath

    neff_dir = trace_dir or tempfile.mkdtemp(prefix="trn_trace_")
    hook = _ntff_hook()
    with hook(neff_dir, [0]):
        results = bass2jax.run_bass_via_pjrt(nc, in_maps, n_cores=len(in_maps))

    ntffs = glob.glob(os.path.join(neff_dir, "*_body*.ntff"))
    if not ntffs:
        print(f"no ntffs in {neff_dir}: {os.listdir(neff_dir)}")
        return bass_utils.BassKernelResults(
            results=results, instructions_and_trace=None,
            profile_json=None, exec_time_ns=None)

    profile = gauge.profiler.Profile(
        profile_path=FishPath(neff_dir),
        kernel_dev_mode=True,
        profile_on_exit=False,
        bass_kernel=nc.m,
        offline_processing=True,
        fname="*_body*",
        metadata={},
    )
    return bass_utils._process_ntff_profile(
        profile, neff_dir, nc, list(range(len(in_maps))),
        None, False, {}, trace_events=False,
    ).as_bass_kernel_results(results)


def kernel(x, Wq, Wk, Wv, Wo, bo, gamma, beta, trace=False):
    global LAST_PROFILE
    x = np.asarray(x, dtype=np.float32)
    Wq, Wk, Wv, Wo = (np.asarray(a, dtype=np.float32) for a in (Wq, Wk, Wv, Wo))
    bo = np.asarray(bo, dtype=np.float32)
    gamma = np.asarray(gamma, dtype=np.float32)
    beta = np.asarray(beta, dtype=np.float32)

    nc = build_nc()
    in_maps = _prep_core_inputs(x, Wq, Wk, Wv, Wo, gamma, beta)
    if trace:
        res = _run_traced(nc, in_maps)
    else:
        res = run_bass_kernel_spmd(nc, in_maps, core_ids=list(range(8)))
    LAST_PROFILE = {"exec_time_ns": res.exec_time_ns}

    B = x.shape[0]
    out = np.empty_like(x)
    for b in range(B):
        acc = x[b] + bo[None, :]
        for g in range(4):
            acc = acc + np.asarray(res.results[b * 4 + g]["partial"],
                                   dtype=np.float32)
        out[b] = acc
    return out


# revision 44
# speedup vs baseline: 1.0205x; 1.0205x over previous
"""Local causal (sliding-window) attention block on 8 TRN2 NeuronCores.

Reference computation (per batch b):
    h = LayerNorm(x) * gamma + beta
    Q = h@Wq, K = h@Wk, V = h@Wv          (heads: 16 x 64)
    S = QK^T/sqrt(dk) masked to causal band of width 256
    out = x + softmax(S)@V @ Wo + bo

Sharding: 8 cores = 2 batches x 4 head-groups (4 heads each).
Each core computes LN(x_b), its head-group's Q/K/V, banded attention,
and a partial out-projection  attn_g @ Wo[g]  (token-major, [T, D]).
Host reduces: out[b] = x[b] + sum_g partial[b,g] + bo.

Implementation notes (v2, overhead-optimized):
- All matmuls run in bf16 (fp32 PSUM accumulation).
- h^T is produced by the DMA XBAR transpose (dma_start(transpose=True)),
  eliminating all PE transposes.
- Attention computes S^T[k, q] tiles directly (k on partitions), so the
  probability tiles feed P@V without any transpose; the softmax
  denominator comes from a ones-column appended to V, and the final
  1/den scaling uses a stride-0 DMA broadcast + one Pool multiply.
- Elementwise work is spread across DVE / Act / Pool to keep the PE the
  only near-saturated engine.
"""

import os

import numpy as np

import concourse.bass as bass
import concourse.tile as tile
from concourse import bacc, mybir
from concourse.bass_utils import run_bass_kernel_spmd

F32 = mybir.dt.float32
F32R = mybir.dt.float32r
BF16 = mybir.dt.bfloat16

T = 2048          # tokens per batch
D = 1024          # model dim
HG = 4            # heads per core
DK = 64           # head dim
DG = HG * DK      # head-group feature width (256)
WIN = 256         # attention window
P = 128           # partitions
NT = T // P       # 16 token tiles
KC = D // P       # 8 feature chunks
NG = NT // 4      # 4 query groups of 512 tokens
LN_EPS = 1e-5
MASKVAL = -1e9

# filled by test.py via run(trace=True)
LAST_PROFILE = {}


def _nq(kb):
    return min(3 * P, (NT - kb) * P)


def _body(tc):
    nc = tc.nc

    x = nc.dram_tensor("x", [T, D], BF16, kind="ExternalInput").ap()
    wq = nc.dram_tensor("wq", [P, KC, DG], BF16, kind="ExternalInput").ap()
    wk = nc.dram_tensor("wk", [P, KC, DG], BF16, kind="ExternalInput").ap()
    wv = nc.dram_tensor("wv", [P, KC, DG], BF16, kind="ExternalInput").ap()
    wo = nc.dram_tensor("wo", [P, DG // P, D], BF16, kind="ExternalInput").ap()
    bq = nc.dram_tensor("bq", [P, DG // P], F32, kind="ExternalInput").ap()
    bk = nc.dram_tensor("bk", [P, DG // P], F32, kind="ExternalInput").ap()
    mc = nc.dram_tensor("mc", [P, HG, 2 * P], BF16, kind="ExternalInput").ap()
    zz = nc.dram_tensor("zz", [1, DK + 1 + 512], BF16, kind="ExternalInput").ap()
    cr = nc.dram_tensor("cr", [1, P + DG], BF16, kind="ExternalInput").ap()
    vones = nc.dram_tensor("vones", [P, NT, HG], BF16, kind="ExternalInput").ap()
    partial = nc.dram_tensor("partial", [T, D], BF16, kind="ExternalOutput").ap()
    dbg = os.environ.get("KDEBUG", "") == "1"
    dscr = nc.dram_tensor("dscr", [NG * 2, 1024], F32,
                          kind="ExternalOutput" if dbg else "Internal").ap()
    dscr2 = nc.dram_tensor("dscr2", [NG * 2, 1024], F32, kind="Internal").ap()
    if dbg:
        d_ht = nc.dram_tensor("d_ht", [P, KC, T], BF16, kind="ExternalOutput").ap()
        d_qt = nc.dram_tensor("d_qt", [P, 2, T], BF16, kind="ExternalOutput").ap()
        d_kt = nc.dram_tensor("d_kt", [P, 2, T], BF16, kind="ExternalOutput").ap()
        d_v = nc.dram_tensor("d_v", [P, NT, HG * (DK + 1)], BF16, kind="ExternalOutput").ap()
        d_ot = nc.dram_tensor("d_ot", [P, 2, T], BF16, kind="ExternalOutput").ap()

    with (
        tc.tile_pool(name="consts", bufs=1) as consts,
        tc.tile_pool(name="big", bufs=1) as big,
        tc.tile_pool(name="sp", bufs=2, space="PSUM") as sp,
        tc.tile_pool(name="ep", bufs=10) as ep,
    ):
        # ---- resident SBUF tensors ----
        wq_sb = consts.tile([P, KC, DG], BF16, tag="wq")
        wk_sb = consts.tile([P, KC, DG], BF16, tag="wk")
        wv_sb = consts.tile([P, KC, DG], BF16, tag="wv")
        wo_sb = consts.tile([P, DG // P, D], BF16, tag="wo")
        bq_sb = consts.tile([P, DG // P], F32, tag="bq")
        bk_sb = consts.tile([P, DG // P], F32, tag="bk")
        mc_sb = consts.tile([P, HG, 2 * P], BF16, tag="mc")
        zz_sb = consts.tile([1, DK + 1 + 512], BF16, tag="zz")
        cr_sb = consts.tile([1, P + DG], BF16, tag="cr")
        eps_sb = consts.tile([P, 1], F32, tag="eps")

        nc.vector.memset(eps_sb, LN_EPS)

        # h^T (feature-major), Q^T/K^T (feature-major), V (token-major,
        # with a ones column per head for the softmax denominator),
        # O^T (attention output, feature-major)
        ht_sb = big.tile([P, KC, T], BF16, tag="ht")
        qt_sb = big.tile([P, DG // P, T], BF16, tag="qt")
        kt_sb = big.tile([P, DG // P, T], BF16, tag="kt")
        v_sb = big.tile([P, NT, HG * (DK + 1)], BF16, tag="v")
        ot_sb = big.tile([P, DG // P, T], BF16, tag="ot")

        # ones columns of V (denominator trick)
        nc.sync.dma_start(out=v_sb[:, :, DK::DK + 1], in_=vones)

        # scores PSUM + exp-tile pools live across front and attention so
        # the first score quads can fill front-phase bubbles
        et_ref = {}

        def st_quad(kb):
            """S^T + exp + band mask for all 4 heads of key block kb."""
            nq = _nq(kb)
            ks = slice(kb * P, (kb + 1) * P)
            et = ep.tile([P, HG, 3 * P], BF16, tag="et")
            for j in range(2):
                s2 = sp.tile([P, 2, 512], F32, tag="s2")
                for hh in range(2):
                    h = 2 * j + hh
                    p0 = (h % 2) * DK
                    nc.tensor.matmul(
                        s2[:, hh, 0:nq],
                        kt_sb[p0:p0 + DK, h // 2, ks],
                        qt_sb[p0:p0 + DK, h // 2, kb * P:kb * P + nq],
                        start=True, stop=True,
                    )
                nc.scalar.activation(
                    out=et[:, 2 * j:2 * j + 2, 0:nq], in_=s2[:, :, 0:nq],
                    func=mybir.ActivationFunctionType.Exp,
                )
            # band mask: diag ([0:128)) and far ([256:384)) 0/1 windows
            nc.vector.tensor_mul(
                et[:, :, 0:P], et[:, :, 0:P], mc_sb[:, :, 0:P])
            if nq == 3 * P:
                nc.vector.tensor_mul(
                    et[:, :, 2 * P:3 * P], et[:, :, 2 * P:3 * P],
                    mc_sb[:, :, P:2 * P])
            et_ref[kb] = et

        # ============ Front: LayerNorm + h^T + Q/K/V projections ============
        x_sb = big.tile([P, NT, D], BF16, tag="x")
        xr = x.rearrange("(n p) d -> p n d", p=P)
        nc.sync.dma_start(out=x_sb[:, 0:2, :], in_=xr[:, 0:2, :])
        nc.sync.dma_start(out=wv_sb, in_=wv)
        nc.sync.dma_start(out=cr_sb, in_=cr)
        nc.sync.dma_start(out=x_sb[:, 2:4, :], in_=xr[:, 2:4, :])
        nc.sync.dma_start(out=wq_sb, in_=wq)
        nc.sync.dma_start(out=wk_sb, in_=wk)
        nc.sync.dma_start(out=bq_sb, in_=bq)
        nc.sync.dma_start(out=bk_sb, in_=bk)
        nc.sync.dma_start(out=mc_sb, in_=mc)
        nc.sync.dma_start(out=zz_sb, in_=zz)
        for q in range(2, 8):
            nc.sync.dma_start(
                out=x_sb[:, 2 * q:2 * (q + 1), :], in_=xr[:, 2 * q:2 * (q + 1), :])
        nc.sync.dma_start(out=wo_sb, in_=wo)
        with (
            tc.tile_pool(name="hp", bufs=3) as hp,
            tc.tile_pool(name="lnst", bufs=4) as lnst,
            tc.tile_pool(name="qkp", bufs=2, space="PSUM") as qkp,
            tc.tile_pool(name="vp", bufs=2, space="PSUM") as vp,
        ):
            front_state = {}
            for tb in range(NT):
                xt = x_sb[:, tb, :]

                j4 = tb % 2
                if j4 == 0:
                    mv4 = lnst.tile([P, 2, 2], F32, tag="mv4")
                    rstd4 = lnst.tile([P, 2], F32, tag="rstd4")
                    front_state.update(mv4=mv4, rstd4=rstd4)
                mv4 = front_state["mv4"]
                rstd4 = front_state["rstd4"]
                stats = lnst.tile([P, 2, 6], F32, tag="stats")
                xg = xt.rearrange("p (g d) -> p g d", g=2)
                nc.vector.bn_stats(out=stats[:, 0, :], in_=xg[:, 0, :])
                nc.vector.bn_stats(out=stats[:, 1, :], in_=xg[:, 1, :])
                nc.vector.bn_aggr(out=mv4[:, j4, :], in_=stats)
                if j4 == 1:
                    # rstd = exp(-0.5*ln(var+eps)), batched over 4 tiles
                    nc.scalar.activation(
                        out=rstd4, in_=mv4[:, :, 1],
                        func=mybir.ActivationFunctionType.Ln,
                        bias=eps_sb, scale=1.0,
                    )
                    nc.scalar.activation(
                        out=rstd4, in_=rstd4,
                        func=mybir.ActivationFunctionType.Exp,
                        scale=-0.5,
                    )
                    for t2 in range(tb - 1, tb + 1):
                        j2 = t2 % 2
                        rs1 = lnst.tile([P, 1], F32, tag="rs1")
                        nc.vector.tensor_copy(rs1, rstd4[:, j2:j2 + 1])
                        nm1 = lnst.tile([P, 1], F32, tag="nm1")
                        nc.vector.tensor_scalar(
                            out=nm1, in0=mv4[:, j2, 0:1], scalar1=rs1,
                            scalar2=-1.0, op0=mybir.AluOpType.mult,
                            op1=mybir.AluOpType.mult,
                        )
                        hn = hp.tile([P, D], BF16, tag="hn")
                        nc.gpsimd.tensor_scalar(
                            out=hn, in0=x_sb[:, t2, :], scalar1=rs1,
                            scalar2=nm1, op0=mybir.AluOpType.mult,
                            op1=mybir.AluOpType.add,
                        )
                        nc.sync.dma_start(
                            out=ht_sb[:, :, t2 * P:(t2 + 1) * P], in_=hn,
                            transpose=True)

                if tb % 2 != 1:
                    continue
                # V projection for the 2 finished tiles (token-major);
                # bv enters as a rank-1 (ones x bv) accumulation
                for t2 in range(tb - 1, tb + 1):
                    t2s = slice(t2 * P, (t2 + 1) * P)
                    ps = vp.tile([P, DG], F32, tag="psv")
                    for kc in range(KC):
                        nc.tensor.matmul(
                            ps, ht_sb[:, kc, t2s], wv_sb[:, kc, :],
                            start=(kc == 0), stop=False,
                        )
                    nc.tensor.matmul(
                        ps, cr_sb[0:1, 0:P], cr_sb[0:1, P:],
                        start=False, stop=True,
                    )
                    nc.scalar.activation(
                        out=v_sb[:, t2, :].rearrange(
                            "p (h d) -> p h d", d=DK + 1)[:, :, 0:DK],
                        in_=ps.rearrange("p (h d) -> p h d", d=DK),
                        func=mybir.ActivationFunctionType.Identity, scale=1.0)

                # Q^T / K^T per completed 512-token slice
                if tb % 4 != 3:
                    continue
                if True:
                    sl = tb // 4
                    ss = slice(sl * 512, (sl + 1) * 512)
                    for w_sb, b_sb, dst in ((wq_sb, bq_sb, qt_sb),
                                            (wk_sb, bk_sb, kt_sb)):
                        for oc in range(DG // P):
                            pq = qkp.tile([P, 512], F32, tag="psqk")
                            for kc in range(KC):
                                nc.tensor.matmul(
                                    pq,
                                    w_sb[:, kc, oc * P:(oc + 1) * P],
                                    ht_sb[:, kc, ss],
                                    start=(kc == 0), stop=(kc == KC - 1),
                                )
                            nc.vector.tensor_scalar_add(
                                dst[:, oc, ss], pq, b_sb[:, oc:oc + 1])

        # ============ Attention (S^T formulation) + out-projection ============
        with (
            tc.tile_pool(name="pvop", bufs=4, space="PSUM") as pvop,
            tc.tile_pool(name="rp", bufs=4) as rp,
            tc.tile_pool(name="bp", bufs=3) as bp,
            tc.tile_pool(name="op", bufs=3) as op,
        ):
            def pv_head(g, h, den2, j):
                """P@V for one head; den row lands in half j of den2."""
                q0 = g * 512
                oc, hh = h // 2, h % 2
                av = pvop.tile([P, 512], F32, tag="po")
                nc.tensor.matmul(
                    av[0:DK + 1, :], zz_sb[0:1, 0:DK + 1], zz_sb[0:1, DK + 1:],
                    start=True, stop=False, skip_group_check=True,
                )
                segs = []
                for kb in range(max(0, 4 * g - 2), 4 * g + 4):
                    a = max(kb * P, q0)
                    b2 = min(kb * P + _nq(kb), q0 + 512)
                    segs.append((kb, a - q0, b2 - q0))
                for i, (kb, a, b2) in enumerate(segs):
                    nc.tensor.matmul(
                        av[0:DK + 1, a:b2],
                        v_sb[:, kb, h * (DK + 1):(h + 1) * (DK + 1)],
                        et_ref[kb][:, h, q0 + a - kb * P:q0 + b2 - kb * P],
                        start=False, stop=(i == len(segs) - 1),
                        skip_group_check=True,
                    )
                nc.scalar.activation(
                    out=den2[0:1, j * 512:(j + 1) * 512], in_=av[DK:DK + 1, :],
                    func=mybir.ActivationFunctionType.Identity, scale=1.0)
                return av

            def recip_pair(g, pair):
                """1/den for two heads: round-trip via DRAM so the
                reciprocal runs partition-parallel ([1,1024]->[128,8])."""
                i = g * 2 + pair
                den2 = recip_pair.den2
                w1 = nc.sync.dma_start(out=dscr[i:i + 1, :], in_=den2)
                tc.chain_iter_dep(f"dw{i}", w1.ins)
                dr = rp.tile([P, 8], F32, tag="dr")
                r1 = nc.sync.dma_start(
                    out=dr, in_=dscr[i:i + 1, :].rearrange(
                        "o (p j) -> (o p) j", p=P))
                tc.chain_iter_dep(f"dw{i}", r1.ins)
                nc.vector.reciprocal(out=dr, in_=dr)
                w2 = nc.sync.dma_start(
                    out=dscr2[i:i + 1, :].rearrange("o (p j) -> (o p) j", p=P),
                    in_=dr)
                tc.chain_iter_dep(f"db{i}", w2.ins)
                bc = bp.tile([DK, 2, 512], F32, tag="bc")
                r2 = nc.sync.dma_start(
                    out=bc, in_=dscr2[i:i + 1, :].rearrange(
                        "o (j q) -> o j q", j=2).to_broadcast([DK, 2, 512]))
                tc.chain_iter_dep(f"db{i}", r2.ins)
                return bc

            def norm_head(g, h, av, bc, j):
                q0 = g * 512
                oc, hh = h // 2, h % 2
                nc.vector.tensor_mul(
                    ot_sb[hh * DK:(hh + 1) * DK, oc, q0:q0 + 512],
                    av[0:DK, :], bc[:, j, :])

            def outproj(tb):
                ts = slice(tb * P, (tb + 1) * P)
                ob = op.tile([P, D], BF16, tag="ob")
                for on in range(2):
                    po = pvop.tile([P, 512], F32, tag="po")
                    for kd in range(DG // P):
                        nc.tensor.matmul(
                            po,
                            ot_sb[:, kd, ts],
                            wo_sb[:, kd, on * 512:(on + 1) * 512],
                            start=(kd == 0), stop=(kd == DG // P - 1),
                        )
                    if (tb + on) % 2 == 0:
                        nc.scalar.activation(
                            out=ob[:, on * 512:(on + 1) * 512], in_=po,
                            func=mybir.ActivationFunctionType.Identity,
                            scale=1.0)
                    else:
                        nc.vector.tensor_copy(
                            ob[:, on * 512:(on + 1) * 512], po)
                nc.sync.dma_start(out=partial[ts, :], in_=ob)

            for kb in range(4):
                st_quad(kb)
            for g in range(NG):
                nxt = [4 * (g + 1) + j for j in range(4)] if g + 1 < NG else []
                prv = list(range(4 * (g - 1), 4 * g)) if g > 0 else []
                den2a = rp.tile([1, 1024], F32, tag="den2")
                recip_pair.den2 = den2a
                av0 = pv_head(g, 0, den2a, 0)
                av1 = pv_head(g, 1, den2a, 1)
                bca = recip_pair(g, 0)
                if nxt:
                    st_quad(nxt[0])
                    st_quad(nxt[1])
                if prv:
                    outproj(prv[0])
                    outproj(prv[1])
                den2b = rp.tile([1, 1024], F32, tag="den2")
                recip_pair.den2 = den2b
                av2 = pv_head(g, 2, den2b, 0)
                av3 = pv_head(g, 3, den2b, 1)
                bcb = recip_pair(g, 1)
                if nxt:
                    st_quad(nxt[2])
                    st_quad(nxt[3])
                norm_head(g, 0, av0, bca, 0)
                norm_head(g, 1, av1, bca, 1)
                if prv:
                    outproj(prv[2])
                    outproj(prv[3])
                norm_head(g, 2, av2, bcb, 0)
                norm_head(g, 3, av3, bcb, 1)
            for tb in range(4 * (NG - 1), NT):
                outproj(tb)

            if dbg:
                nc.sync.dma_start(out=d_ht, in_=ht_sb)
                nc.sync.dma_start(out=d_qt, in_=qt_sb)
                nc.sync.dma_start(out=d_kt, in_=kt_sb)
                nc.sync.dma_start(out=d_v, in_=v_sb)
                nc.sync.dma_start(out=d_ot, in_=ot_sb)


def build_nc():
    nc = bacc.Bacc("TRN2", target_bir_lowering=False, debug=False,
                   num_devices=8)
    with tile.TileContext(nc) as tc:
        _body(tc)
    nc.compile()
    return nc


def _prep_core_inputs(x, Wq, Wk, Wv, Wo, gamma, beta):
    """Host-side prep: per-(batch, head-group) input dicts."""
    import ml_dtypes
    BF = ml_dtypes.bfloat16
    B = x.shape[0]
    kk = np.arange(P)[:, None]
    qq = np.arange(P)[None, :]
    md = (kk <= qq).astype(BF)
    mf = (kk > qq).astype(BF)
    m1 = np.concatenate([md, mf], axis=1)
    mcomb = np.ascontiguousarray(np.stack([m1] * 4, axis=1))

    def fold(w):
        # [D, DG] -> [128, KC, DG] with d = c*128 + p
        return np.ascontiguousarray(
            w.reshape(KC, P, DG).transpose(1, 0, 2)).astype(BF)

    in_maps = []
    for b in range(B):
        for g in range(4):
            sl = slice(g * DG, (g + 1) * DG)
            sq = np.float32(1.0 / np.sqrt(DK))
            wq_g = fold(gamma[:, None] * Wq[:, sl] * sq)
            wk_g = fold(gamma[:, None] * Wk[:, sl])
            wv_g = fold(gamma[:, None] * Wv[:, sl])
            wo_g = np.ascontiguousarray(
                Wo[sl, :].reshape(DG // P, P, D).transpose(1, 0, 2)).astype(BF)
            bq_g = ((beta @ Wq[:, sl]) * sq).astype(np.float32)
            bk_g = (beta @ Wk[:, sl]).astype(np.float32)
            bv_g = (beta @ Wv[:, sl]).astype(np.float32)
            crow = np.concatenate(
                [np.ones(P, dtype=np.float32), bv_g]).astype(BF)
            in_maps.append({
                "x": np.ascontiguousarray(x[b]).astype(BF),
                "wq": wq_g, "wk": wk_g, "wv": wv_g, "wo": wo_g,
                "bq": np.ascontiguousarray(bq_g.reshape(DG // P, P).T),
                "bk": np.ascontiguousarray(bk_g.reshape(DG // P, P).T),
                "cr": crow[None, :],
                "mc": mcomb,
                "zz": np.zeros((1, DK + 1 + 512), dtype=BF),
                "vones": np.ones((P, NT, HG), dtype=BF),
            })
    return in_maps


def _ntff_hook(so_path="/opt/axon/libaxon_pjrt.so"):
    import contextlib
    import ctypes

    lib = ctypes.CDLL(so_path)
    lib.axon_start_nrt_profile.argtypes = [
        ctypes.POINTER(ctypes.c_int64), ctypes.c_size_t]
    lib.axon_start_nrt_profile.restype = ctypes.c_int64
    lib.axon_stop_nrt_profile.argtypes = [ctypes.c_char_p]
    lib.axon_stop_nrt_profile.restype = ctypes.c_int64

    @contextlib.contextmanager
    def _hook(output_dir, device_ids):
        import jax
        jax.devices()
        if device_ids:
            ids = (ctypes.c_int64 * len(device_ids))(*device_ids)
            rc = lib.axon_start_nrt_profile(ids, len(device_ids))
        else:
            rc = lib.axon_start_nrt_profile(None, 0)
        if rc != 0:
            raise RuntimeError(f"axon_start_nrt_profile rc={rc}")
        try:
            yield
        finally:
            n = lib.axon_stop_nrt_profile(str(output_dir).encode())
            print(f"profile: {n} file(s) written to {output_dir}")

    return _hook


def _run_traced(nc, in_maps, trace_dir=None):
    """Execute via PJRT with NTFF capture; return BassKernelResults with
    exec_time_ns and a perfetto trace."""
    import glob
    import tempfile

    import gauge.profiler
    from concourse import bass2jax, bass_utils
    from concourse._compat import FishPath

    neff_dir = trace_dir or tempfile.mkdtemp(prefix="trn_trace_")
    hook = _ntff_hook()
    with hook(neff_dir, [0]):
        results = bass2jax.run_bass_via_pjrt(nc, in_maps, n_cores=len(in_maps))

    ntffs = glob.glob(os.path.join(neff_dir, "*_body*.ntff"))
    if not ntffs:
        print(f"no ntffs in {neff_dir}: {os.listdir(neff_dir)}")
        return bass_utils.BassKernelResults(
            results=results, instructions_and_trace=None,
            profile_json=None, exec_time_ns=None)

    profile = gauge.profiler.Profile(
        profile_path=FishPath(neff_dir),
        kernel_dev_mode=True,
        profile_on_exit=False,
        bass_kernel=nc.m,
        offline_processing=True,
        fname="*_body*",
        metadata={},
    )
    return bass_utils._process_ntff_profile(
        profile, neff_dir, nc, list(range(len(in_maps))),
        None, False, {}, trace_events=False,
    ).as_bass_kernel_results(results)


def kernel(x, Wq, Wk, Wv, Wo, bo, gamma, beta, trace=False):
    global LAST_PROFILE
    x = np.asarray(x, dtype=np.float32)
    Wq, Wk, Wv, Wo = (np.asarray(a, dtype=np.float32) for a in (Wq, Wk, Wv, Wo))
    bo = np.asarray(bo, dtype=np.float32)
    gamma = np.asarray(gamma, dtype=np.float32)
    beta = np.asarray(beta, dtype=np.float32)

    nc = build_nc()
    in_maps = _prep_core_inputs(x, Wq, Wk, Wv, Wo, gamma, beta)
    if trace:
        res = _run_traced(nc, in_maps)
    else:
        res = run_bass_kernel_spmd(nc, in_maps, core_ids=list(range(8)))
    LAST_PROFILE = {"exec_time_ns": res.exec_time_ns}

    B = x.shape[0]
    out = np.empty_like(x)
    for b in range(B):
        acc = x[b] + bo[None, :]
        for g in range(4):
            acc = acc + np.asarray(res.results[b * 4 + g]["partial"],
                                   dtype=np.float32)
        out[b] = acc
    return out


# revision 45
# speedup vs baseline: 1.0284x; 1.0078x over previous
"""Local causal (sliding-window) attention block on 8 TRN2 NeuronCores.

Reference computation (per batch b):
    h = LayerNorm(x) * gamma + beta
    Q = h@Wq, K = h@Wk, V = h@Wv          (heads: 16 x 64)
    S = QK^T/sqrt(dk) masked to causal band of width 256
    out = x + softmax(S)@V @ Wo + bo

Sharding: 8 cores = 2 batches x 4 head-groups (4 heads each).
Each core computes LN(x_b), its head-group's Q/K/V, banded attention,
and a partial out-projection  attn_g @ Wo[g]  (token-major, [T, D]).
Host reduces: out[b] = x[b] + sum_g partial[b,g] + bo.

Implementation notes (v2, overhead-optimized):
- All matmuls run in bf16 (fp32 PSUM accumulation).
- h^T is produced by the DMA XBAR transpose (dma_start(transpose=True)),
  eliminating all PE transposes.
- Attention computes S^T[k, q] tiles directly (k on partitions), so the
  probability tiles feed P@V without any transpose; the softmax
  denominator comes from a ones-column appended to V, and the final
  1/den scaling uses a stride-0 DMA broadcast + one Pool multiply.
- Elementwise work is spread across DVE / Act / Pool to keep the PE the
  only near-saturated engine.
"""

import os

import numpy as np

import concourse.bass as bass
import concourse.tile as tile
from concourse import bacc, mybir
from concourse.bass_utils import run_bass_kernel_spmd

F32 = mybir.dt.float32
F32R = mybir.dt.float32r
BF16 = mybir.dt.bfloat16

T = 2048          # tokens per batch
D = 1024          # model dim
HG = 4            # heads per core
DK = 64           # head dim
DG = HG * DK      # head-group feature width (256)
WIN = 256         # attention window
P = 128           # partitions
NT = T // P       # 16 token tiles
KC = D // P       # 8 feature chunks
NG = NT // 4      # 4 query groups of 512 tokens
LN_EPS = 1e-5
MASKVAL = -1e9

# filled by test.py via run(trace=True)
LAST_PROFILE = {}


def _nq(kb):
    return min(3 * P, (NT - kb) * P)


def _body(tc):
    nc = tc.nc

    x = nc.dram_tensor("x", [T, D], BF16, kind="ExternalInput").ap()
    wq = nc.dram_tensor("wq", [P, KC, DG], BF16, kind="ExternalInput").ap()
    wk = nc.dram_tensor("wk", [P, KC, DG], BF16, kind="ExternalInput").ap()
    wv = nc.dram_tensor("wv", [P, KC, DG], BF16, kind="ExternalInput").ap()
    wo = nc.dram_tensor("wo", [P, DG // P, D], BF16, kind="ExternalInput").ap()
    bq = nc.dram_tensor("bq", [P, DG // P], F32, kind="ExternalInput").ap()
    bk = nc.dram_tensor("bk", [P, DG // P], F32, kind="ExternalInput").ap()
    mc = nc.dram_tensor("mc", [P, HG, 2 * P], BF16, kind="ExternalInput").ap()
    zz = nc.dram_tensor("zz", [1, DK + 1 + 512], BF16, kind="ExternalInput").ap()
    cr = nc.dram_tensor("cr", [1, P + DG], BF16, kind="ExternalInput").ap()
    vones = nc.dram_tensor("vones", [P, NT, HG], BF16, kind="ExternalInput").ap()
    partial = nc.dram_tensor("partial", [T, D], BF16, kind="ExternalOutput").ap()
    dbg = os.environ.get("KDEBUG", "") == "1"
    dscr = nc.dram_tensor("dscr", [NG * 2, 1024], F32,
                          kind="ExternalOutput" if dbg else "Internal").ap()
    dscr2 = nc.dram_tensor("dscr2", [NG * 2, 1024], F32, kind="Internal").ap()
    if dbg:
        d_ht = nc.dram_tensor("d_ht", [P, KC, T], BF16, kind="ExternalOutput").ap()
        d_qt = nc.dram_tensor("d_qt", [P, 2, T], BF16, kind="ExternalOutput").ap()
        d_kt = nc.dram_tensor("d_kt", [P, 2, T], BF16, kind="ExternalOutput").ap()
        d_v = nc.dram_tensor("d_v", [P, NT, HG * (DK + 1)], BF16, kind="ExternalOutput").ap()
        d_ot = nc.dram_tensor("d_ot", [P, 2, T], BF16, kind="ExternalOutput").ap()

    with (
        tc.tile_pool(name="consts", bufs=1) as consts,
        tc.tile_pool(name="big", bufs=1) as big,
        tc.tile_pool(name="sp", bufs=2, space="PSUM") as sp,
        tc.tile_pool(name="ep", bufs=10) as ep,
    ):
        # ---- resident SBUF tensors ----
        wq_sb = consts.tile([P, KC, DG], BF16, tag="wq")
        wk_sb = consts.tile([P, KC, DG], BF16, tag="wk")
        wv_sb = consts.tile([P, KC, DG], BF16, tag="wv")
        wo_sb = consts.tile([P, DG // P, D], BF16, tag="wo")
        bq_sb = consts.tile([P, DG // P], F32, tag="bq")
        bk_sb = consts.tile([P, DG // P], F32, tag="bk")
        mc_sb = consts.tile([P, HG, 2 * P], BF16, tag="mc")
        zz_sb = consts.tile([1, DK + 1 + 512], BF16, tag="zz")
        cr_sb = consts.tile([1, P + DG], BF16, tag="cr")
        eps_sb = consts.tile([P, 1], F32, tag="eps")

        nc.vector.memset(eps_sb, LN_EPS)

        # h^T (feature-major), Q^T/K^T (feature-major), V (token-major,
        # with a ones column per head for the softmax denominator),
        # O^T (attention output, feature-major)
        ht_sb = big.tile([P, KC, T], BF16, tag="ht")
        qt_sb = big.tile([P, DG // P, T], BF16, tag="qt")
        kt_sb = big.tile([P, DG // P, T], BF16, tag="kt")
        v_sb = big.tile([P, NT, HG * (DK + 1)], BF16, tag="v")
        ot_sb = big.tile([P, DG // P, T], BF16, tag="ot")

        # ones columns of V (denominator trick)
        nc.scalar.dma_start(out=v_sb[:, :, DK::DK + 1], in_=vones)

        # scores PSUM + exp-tile pools live across front and attention so
        # the first score quads can fill front-phase bubbles
        et_ref = {}

        def st_quad(kb):
            """S^T + exp + band mask for all 4 heads of key block kb."""
            nq = _nq(kb)
            ks = slice(kb * P, (kb + 1) * P)
            et = ep.tile([P, HG, 3 * P], BF16, tag="et")
            for j in range(2):
                s2 = sp.tile([P, 2, 512], F32, tag="s2")
                for hh in range(2):
                    h = 2 * j + hh
                    p0 = (h % 2) * DK
                    nc.tensor.matmul(
                        s2[:, hh, 0:nq],
                        kt_sb[p0:p0 + DK, h // 2, ks],
                        qt_sb[p0:p0 + DK, h // 2, kb * P:kb * P + nq],
                        start=True, stop=True,
                    )
                nc.scalar.activation(
                    out=et[:, 2 * j:2 * j + 2, 0:nq], in_=s2[:, :, 0:nq],
                    func=mybir.ActivationFunctionType.Exp,
                )
            # band mask: diag ([0:128)) and far ([256:384)) 0/1 windows
            nc.vector.tensor_mul(
                et[:, :, 0:P], et[:, :, 0:P], mc_sb[:, :, 0:P])
            if nq == 3 * P:
                nc.vector.tensor_mul(
                    et[:, :, 2 * P:3 * P], et[:, :, 2 * P:3 * P],
                    mc_sb[:, :, P:2 * P])
            et_ref[kb] = et

        # ============ Front: LayerNorm + h^T + Q/K/V projections ============
        x_sb = big.tile([P, NT, D], BF16, tag="x")
        xr = x.rearrange("(n p) d -> p n d", p=P)
        nc.sync.dma_start(out=x_sb[:, 0:2, :], in_=xr[:, 0:2, :])
        nc.sync.dma_start(out=wv_sb, in_=wv)
        nc.sync.dma_start(out=cr_sb, in_=cr)
        nc.scalar.dma_start(out=x_sb[:, 2:4, :], in_=xr[:, 2:4, :])
        nc.scalar.dma_start(out=wq_sb, in_=wq)
        nc.scalar.dma_start(out=wk_sb, in_=wk)
        nc.scalar.dma_start(out=bq_sb, in_=bq)
        nc.scalar.dma_start(out=bk_sb, in_=bk)
        nc.scalar.dma_start(out=mc_sb, in_=mc)
        nc.scalar.dma_start(out=zz_sb, in_=zz)
        for q in range(2, 8):
            nc.scalar.dma_start(
                out=x_sb[:, 2 * q:2 * (q + 1), :], in_=xr[:, 2 * q:2 * (q + 1), :])
        nc.scalar.dma_start(out=wo_sb, in_=wo)
        with (
            tc.tile_pool(name="hp", bufs=3) as hp,
            tc.tile_pool(name="lnst", bufs=4) as lnst,
            tc.tile_pool(name="qkp", bufs=2, space="PSUM") as qkp,
            tc.tile_pool(name="vp", bufs=2, space="PSUM") as vp,
        ):
            front_state = {}
            for tb in range(NT):
                xt = x_sb[:, tb, :]

                j4 = tb % 2
                if j4 == 0:
                    mv4 = lnst.tile([P, 2, 2], F32, tag="mv4")
                    rstd4 = lnst.tile([P, 2], F32, tag="rstd4")
                    front_state.update(mv4=mv4, rstd4=rstd4)
                mv4 = front_state["mv4"]
                rstd4 = front_state["rstd4"]
                stats = lnst.tile([P, 2, 6], F32, tag="stats")
                xg = xt.rearrange("p (g d) -> p g d", g=2)
                nc.vector.bn_stats(out=stats[:, 0, :], in_=xg[:, 0, :])
                nc.vector.bn_stats(out=stats[:, 1, :], in_=xg[:, 1, :])
                nc.vector.bn_aggr(out=mv4[:, j4, :], in_=stats)
                if j4 == 1:
                    # rstd = exp(-0.5*ln(var+eps)), batched over 4 tiles
                    nc.scalar.activation(
                        out=rstd4, in_=mv4[:, :, 1],
                        func=mybir.ActivationFunctionType.Ln,
                        bias=eps_sb, scale=1.0,
                    )
                    nc.scalar.activation(
                        out=rstd4, in_=rstd4,
                        func=mybir.ActivationFunctionType.Exp,
                        scale=-0.5,
                    )
                    for t2 in range(tb - 1, tb + 1):
                        j2 = t2 % 2
                        rs1 = lnst.tile([P, 1], F32, tag="rs1")
                        nc.vector.tensor_copy(rs1, rstd4[:, j2:j2 + 1])
                        nm1 = lnst.tile([P, 1], F32, tag="nm1")
                        nc.vector.tensor_scalar(
                            out=nm1, in0=mv4[:, j2, 0:1], scalar1=rs1,
                            scalar2=-1.0, op0=mybir.AluOpType.mult,
                            op1=mybir.AluOpType.mult,
                        )
                        hn = hp.tile([P, D], BF16, tag="hn")
                        nc.gpsimd.tensor_scalar(
                            out=hn, in0=x_sb[:, t2, :], scalar1=rs1,
                            scalar2=nm1, op0=mybir.AluOpType.mult,
                            op1=mybir.AluOpType.add,
                        )
                        nc.sync.dma_start(
                            out=ht_sb[:, :, t2 * P:(t2 + 1) * P], in_=hn,
                            transpose=True)

                if tb % 2 != 1:
                    continue
                # V projection for the 2 finished tiles (token-major);
                # bv enters as a rank-1 (ones x bv) accumulation
                for t2 in range(tb - 1, tb + 1):
                    t2s = slice(t2 * P, (t2 + 1) * P)
                    ps = vp.tile([P, DG], F32, tag="psv")
                    for kc in range(KC):
                        nc.tensor.matmul(
                            ps, ht_sb[:, kc, t2s], wv_sb[:, kc, :],
                            start=(kc == 0), stop=False,
                        )
                    nc.tensor.matmul(
                        ps, cr_sb[0:1, 0:P], cr_sb[0:1, P:],
                        start=False, stop=True,
                    )
                    nc.scalar.activation(
                        out=v_sb[:, t2, :].rearrange(
                            "p (h d) -> p h d", d=DK + 1)[:, :, 0:DK],
                        in_=ps.rearrange("p (h d) -> p h d", d=DK),
                        func=mybir.ActivationFunctionType.Identity, scale=1.0)

                # Q^T / K^T per completed 512-token slice
                if tb % 4 != 3:
                    continue
                if True:
                    sl = tb // 4
                    ss = slice(sl * 512, (sl + 1) * 512)
                    for w_sb, b_sb, dst in ((wq_sb, bq_sb, qt_sb),
                                            (wk_sb, bk_sb, kt_sb)):
                        for oc in range(DG // P):
                            pq = qkp.tile([P, 512], F32, tag="psqk")
                            for kc in range(KC):
                                nc.tensor.matmul(
                                    pq,
                                    w_sb[:, kc, oc * P:(oc + 1) * P],
                                    ht_sb[:, kc, ss],
                                    start=(kc == 0), stop=(kc == KC - 1),
                                )
                            nc.vector.tensor_scalar_add(
                                dst[:, oc, ss], pq, b_sb[:, oc:oc + 1])

        # ============ Attention (S^T formulation) + out-projection ============
        with (
            tc.tile_pool(name="pvop", bufs=4, space="PSUM") as pvop,
            tc.tile_pool(name="rp", bufs=4) as rp,
            tc.tile_pool(name="bp", bufs=3) as bp,
            tc.tile_pool(name="op", bufs=3) as op,
        ):
            def pv_head(g, h, den2, j):
                """P@V for one head; den row lands in half j of den2."""
                q0 = g * 512
                oc, hh = h // 2, h % 2
                av = pvop.tile([P, 512], F32, tag="po")
                nc.tensor.matmul(
                    av[0:DK + 1, :], zz_sb[0:1, 0:DK + 1], zz_sb[0:1, DK + 1:],
                    start=True, stop=False, skip_group_check=True,
                )
                segs = []
                for kb in range(max(0, 4 * g - 2), 4 * g + 4):
                    a = max(kb * P, q0)
                    b2 = min(kb * P + _nq(kb), q0 + 512)
                    segs.append((kb, a - q0, b2 - q0))
                for i, (kb, a, b2) in enumerate(segs):
                    nc.tensor.matmul(
                        av[0:DK + 1, a:b2],
                        v_sb[:, kb, h * (DK + 1):(h + 1) * (DK + 1)],
                        et_ref[kb][:, h, q0 + a - kb * P:q0 + b2 - kb * P],
                        start=False, stop=(i == len(segs) - 1),
                        skip_group_check=True,
                    )
                nc.scalar.activation(
                    out=den2[0:1, j * 512:(j + 1) * 512], in_=av[DK:DK + 1, :],
                    func=mybir.ActivationFunctionType.Identity, scale=1.0)
                return av

            def recip_pair(g, pair):
                """1/den for two heads: round-trip via DRAM so the
                reciprocal runs partition-parallel ([1,1024]->[128,8])."""
                i = g * 2 + pair
                den2 = recip_pair.den2
                w1 = nc.sync.dma_start(out=dscr[i:i + 1, :], in_=den2)
                tc.chain_iter_dep(f"dw{i}", w1.ins)
                dr = rp.tile([P, 8], F32, tag="dr")
                r1 = nc.sync.dma_start(
                    out=dr, in_=dscr[i:i + 1, :].rearrange(
                        "o (p j) -> (o p) j", p=P))
                tc.chain_iter_dep(f"dw{i}", r1.ins)
                nc.vector.reciprocal(out=dr, in_=dr)
                w2 = nc.sync.dma_start(
                    out=dscr2[i:i + 1, :].rearrange("o (p j) -> (o p) j", p=P),
                    in_=dr)
                tc.chain_iter_dep(f"db{i}", w2.ins)
                bc = bp.tile([DK, 2, 512], F32, tag="bc")
                r2 = nc.sync.dma_start(
                    out=bc, in_=dscr2[i:i + 1, :].rearrange(
                        "o (j q) -> o j q", j=2).to_broadcast([DK, 2, 512]))
                tc.chain_iter_dep(f"db{i}", r2.ins)
                return bc

            def norm_head(g, h, av, bc, j):
                q0 = g * 512
                oc, hh = h // 2, h % 2
                nc.vector.tensor_mul(
                    ot_sb[hh * DK:(hh + 1) * DK, oc, q0:q0 + 512],
                    av[0:DK, :], bc[:, j, :])

            def outproj(tb):
                ts = slice(tb * P, (tb + 1) * P)
                ob = op.tile([P, D], BF16, tag="ob")
                for on in range(2):
                    po = pvop.tile([P, 512], F32, tag="po")
                    for kd in range(DG // P):
                        nc.tensor.matmul(
                            po,
                            ot_sb[:, kd, ts],
                            wo_sb[:, kd, on * 512:(on + 1) * 512],
                            start=(kd == 0), stop=(kd == DG // P - 1),
                        )
                    if (tb + on) % 2 == 0:
                        nc.scalar.activation(
                            out=ob[:, on * 512:(on + 1) * 512], in_=po,
                            func=mybir.ActivationFunctionType.Identity,
                            scale=1.0)
                    else:
                        nc.vector.tensor_copy(
                            ob[:, on * 512:(on + 1) * 512], po)
                nc.gpsimd.dma_start(out=partial[ts, :], in_=ob)

            for kb in range(4):
                st_quad(kb)
            for g in range(NG):
                nxt = [4 * (g + 1) + j for j in range(4)] if g + 1 < NG else []
                prv = list(range(4 * (g - 1), 4 * g)) if g > 0 else []
                den2a = rp.tile([1, 1024], F32, tag="den2")
                recip_pair.den2 = den2a
                av0 = pv_head(g, 0, den2a, 0)
                av1 = pv_head(g, 1, den2a, 1)
                bca = recip_pair(g, 0)
                if nxt:
                    st_quad(nxt[0])
                    st_quad(nxt[1])
                if prv:
                    outproj(prv[0])
                    outproj(prv[1])
                den2b = rp.tile([1, 1024], F32, tag="den2")
                recip_pair.den2 = den2b
                av2 = pv_head(g, 2, den2b, 0)
                av3 = pv_head(g, 3, den2b, 1)
                bcb = recip_pair(g, 1)
                if nxt:
                    st_quad(nxt[2])
                    st_quad(nxt[3])
                norm_head(g, 0, av0, bca, 0)
                norm_head(g, 1, av1, bca, 1)
                if prv:
                    outproj(prv[2])
                    outproj(prv[3])
                norm_head(g, 2, av2, bcb, 0)
                norm_head(g, 3, av3, bcb, 1)
            for tb in range(4 * (NG - 1), NT):
                outproj(tb)

            if dbg:
                nc.sync.dma_start(out=d_ht, in_=ht_sb)
                nc.sync.dma_start(out=d_qt, in_=qt_sb)
                nc.sync.dma_start(out=d_kt, in_=kt_sb)
                nc.sync.dma_start(out=d_v, in_=v_sb)
                nc.sync.dma_start(out=d_ot, in_=ot_sb)


def build_nc():
    nc = bacc.Bacc("TRN2", target_bir_lowering=False, debug=False,
                   num_devices=8)
    with tile.TileContext(nc) as tc:
        _body(tc)
    nc.compile()
    return nc


def _prep_core_inputs(x, Wq, Wk, Wv, Wo, gamma, beta):
    """Host-side prep: per-(batch, head-group) input dicts."""
    import ml_dtypes
    BF = ml_dtypes.bfloat16
    B = x.shape[0]
    kk = np.arange(P)[:, None]
    qq = np.arange(P)[None, :]
    md = (kk <= qq).astype(BF)
    mf = (kk > qq).astype(BF)
    m1 = np.concatenate([md, mf], axis=1)
    mcomb = np.ascontiguousarray(np.stack([m1] * 4, axis=1))

    def fold(w):
        # [D, DG] -> [128, KC, DG] with d = c*128 + p
        return np.ascontiguousarray(
            w.reshape(KC, P, DG).transpose(1, 0, 2)).astype(BF)

    in_maps = []
    for b in range(B):
        for g in range(4):
            sl = slice(g * DG, (g + 1) * DG)
            sq = np.float32(1.0 / np.sqrt(DK))
            wq_g = fold(gamma[:, None] * Wq[:, sl] * sq)
            wk_g = fold(gamma[:, None] * Wk[:, sl])
            wv_g = fold(gamma[:, None] * Wv[:, sl])
            wo_g = np.ascontiguousarray(
                Wo[sl, :].reshape(DG // P, P, D).transpose(1, 0, 2)).astype(BF)
            bq_g = ((beta @ Wq[:, sl]) * sq).astype(np.float32)
            bk_g = (beta @ Wk[:, sl]).astype(np.float32)
            bv_g = (beta @ Wv[:, sl]).astype(np.float32)
            crow = np.concatenate(
                [np.ones(P, dtype=np.float32), bv_g]).astype(BF)
            in_maps.append({
                "x": np.ascontiguousarray(x[b]).astype(BF),
                "wq": wq_g, "wk": wk_g, "wv": wv_g, "wo": wo_g,
                "bq": np.ascontiguousarray(bq_g.reshape(DG // P, P).T),
                "bk": np.ascontiguousarray(bk_g.reshape(DG // P, P).T),
                "cr": crow[None, :],
                "mc": mcomb,
                "zz": np.zeros((1, DK + 1 + 512), dtype=BF),
                "vones": np.ones((P, NT, HG), dtype=BF),
            })
    return in_maps


def _ntff_hook(so_path="/opt/axon/libaxon_pjrt.so"):
    import contextlib
    import ctypes

    lib = ctypes.CDLL(so_path)
    lib.axon_start_nrt_profile.argtypes = [
        ctypes.POINTER(ctypes.c_int64), ctypes.c_size_t]
    lib.axon_start_nrt_profile.restype = ctypes.c_int64
    lib.axon_stop_nrt_profile.argtypes = [ctypes.c_char_p]
    lib.axon_stop_nrt_profile.restype = ctypes.c_int64

    @contextlib.contextmanager
    def _hook(output_dir, device_ids):
        import jax
        jax.devices()
        if device_ids:
            ids = (ctypes.c_int64 * len(device_ids))(*device_ids)
            rc = lib.axon_start_nrt_profile(ids, len(device_ids))
        else:
            rc = lib.axon_start_nrt_profile(None, 0)
        if rc != 0:
            raise RuntimeError(f"axon_start_nrt_profile rc={rc}")
        try:
            yield
        finally:
            n = lib.axon_stop_nrt_profile(str(output_dir).encode())
            print(f"profile: {n} file(s) written to {output_dir}")

    return _hook


def _run_traced(nc, in_maps, trace_dir=None):
    """Execute via PJRT with NTFF capture; return BassKernelResults with
    exec_time_ns and a perfetto trace."""
    import glob
    import tempfile

    import gauge.profiler
    from concourse import bass2jax, bass_utils
    from concourse._compat import FishPath

    neff_dir = trace_dir or tempfile.mkdtemp(prefix="trn_trace_")
    hook = _ntff_hook()
    with hook(neff_dir, [0]):
        results = bass2jax.run_bass_via_pjrt(nc, in_maps, n_cores=len(in_maps))

    ntffs = glob.glob(os.path.join(neff_dir, "*_body*.ntff"))
    if not ntffs:
        print(f"no ntffs in {neff_dir}: {os.listdir(neff_dir)}")
        return bass_utils.BassKernelResults(
            results=results, instructions_and_trace=None,
            profile_json=None, exec_time_ns=None)

    profile = gauge.profiler.Profile(
        profile_path=FishPath(neff_dir),
        kernel_dev_mode=True,
        profile_on_exit=False,
        bass_kernel=nc.m,
        offline_processing=True,
        fname="*_body*",
        metadata={},
    )
    return bass_utils._process_ntff_profile(
        profile, neff_dir, nc, list(range(len(in_maps))),
        None, False, {}, trace_events=False,
    ).as_bass_kernel_results(results)


def kernel(x, Wq, Wk, Wv, Wo, bo, gamma, beta, trace=False):
    global LAST_PROFILE
    x = np.asarray(x, dtype=np.float32)
    Wq, Wk, Wv, Wo = (np.asarray(a, dtype=np.float32) for a in (Wq, Wk, Wv, Wo))
    bo = np.asarray(bo, dtype=np.float32)
    gamma = np.asarray(gamma, dtype=np.float32)
    beta = np.asarray(beta, dtype=np.float32)

    nc = build_nc()
    in_maps = _prep_core_inputs(x, Wq, Wk, Wv, Wo, gamma, beta)
    if trace:
        res = _run_traced(nc, in_maps)
    else:
        res = run_bass_kernel_spmd(nc, in_maps, core_ids=list(range(8)))
    LAST_PROFILE = {"exec_time_ns": res.exec_time_ns}

    B = x.shape[0]
    out = np.empty_like(x)
    for b in range(B):
        acc = x[b] + bo[None, :]
        for g in range(4):
            acc = acc + np.asarray(res.results[b * 4 + g]["partial"],
                                   dtype=np.float32)
        out[b] = acc
    return out
